# revision 1
# baseline (speedup 1.0000x reference)
"""Euler characteristic curve (cubical complex) kernel for Trainium2.

Problem: x [32,16,128,128] f32 -> ECC [32,16,64] f32.
Per (b,c) slice: every cell of the 255x255 vertex-mode cubical grid has
filtration value = max over its corner vertices, bin K = ceil(63*F) in [0,63],
ECC(t) = #V(K<=t) - #Eh(K<=t) - #Ev(K<=t) + #Q(K<=t)  (cumulative counts).

Strategy (per core, 64 slices, pure data parallel over 8 cores):
 - pack 8 slices per SBUF group: partition p = (slice s=p//16, block hp=p%16),
   each partition holds 9 overlapping image rows (8*hp .. 8*hp+8) so all
   neighbor maxes are free-dim shifts (DVE lanes cannot cross partitions).
 - exact binning: y=63*x, Ki=int cast, K = cast_back(Ki) + (y > cast_back);
   exact ceil under either truncating or round-to-nearest cast semantics.
 - cell bins via bf16 neighbor maxes (x->63x->ceil monotone: bin(max)=max(bins)).
 - per threshold t: ONE fused scalar_tensor_tensor pass over the packed cell
   array [V|Q|Eh|Ev] = [0:4096]: (K is_le t) * sign, accum_out = per-partition
   signed count (= chi contribution). Sentinel 64.0 marks pad columns /
   phantom boundary cells (never <= 63). Exact integer arithmetic throughout.
 - per-slice partition reduction via one PE matmul with block-ones weights,
   accumulated across groups in PSUM; single [64,64] store at the end.
 - host: ECC = device counts directly.

Toolchain notes: this container's walrus rejects >1 sync wait per instruction
(_legalize_waits splits them onto NoOps); fused accum ops run at DVE 1x mode
(~1 elem/lane/cycle) which sets the ~2.3ms/core floor for the 64 threshold
passes (64 x 65025 cells x 8 slices per partition-row group).
"""

import numpy as np

B, C, H, W = 32, 16, 128, 128
RES = 64
NCORES = 8
SLICES = B * C                  # 512
SPC = SLICES // NCORES          # 64 slices per core
SLG = 8                         # slices per group
GROUPS = SPC // SLG             # 8 groups
NPART = 128
NROWS = SPC * H                 # 8192 rows per core
TSPLIT = 30                     # thresholds on DVE; rest on ACT
PAD_ROWS = 8                    # host pads input to 8200 rows

# kcat free-dim layout (bf16)
VOFF, QOFF, EHOFF, EVOFF = 0, 1024, 2048, 3072
KAUX, EHAUX = 4096, 4224        # K row 8, Eh row 8 (aux, not counted)
KCAT_W = 4352

_CACHE = {}


def _build_program(legalize: bool = True):
    import concourse.bass as bass
    import concourse.mybir as mybir
    from concourse.tile import TileContext
    from concourse.alu_op_type import AluOpType as alu

    dt = mybir.dt
    nc = bass.Bass("TRN2", target_bir_lowering=False, debug=False)

    # host-packed input: per group-pair/partition, two slabs of (8 owned image
    # rows + the overlap row); slice-boundary phantoms pre-set to 2.0
    x_dram = nc.dram_tensor(
        "xi", [GROUPS // 2, NPART, 2304], dt.float32, kind="ExternalInput"
    ).ap()
    # column-shifted block-ones: bselw[p, m] = 1 iff m == 56 + p//16, so that
    # bselw[:, 56-8g : 120-8g][p, po] == 1 iff po == 8g + p//16
    bsel_dram = nc.dram_tensor(
        "bsel", [NPART, 120], dt.float32, kind="ExternalInput"
    ).ap()
    out_dram = nc.dram_tensor("cnt", [SPC, 2 * RES], dt.float32, kind="ExternalOutput").ap()

    with TileContext(nc) as tc:
        with (
            tc.tile_pool(name="cst", bufs=1) as cpool,
            tc.tile_pool(name="xfp", bufs=GROUPS // 2) as xfpool,
            tc.tile_pool(name="cntp", bufs=2) as cntpool,
            tc.tile_pool(name="ps", bufs=1, space="PSUM") as pspool,
        ):
            # block-ones weights for per-slice partition reduction
            blocksel = cpool.tile([NPART, 120], dt.float32)
            nc.sync.dma_start(blocksel[:, :], bsel_dram)
            # absorb the blocksel-DMA wait on the PE engine clock so later
            # matmuls carry only a single (DVE) wait
            dummy = pspool.tile([SPC, 8], dt.float32)
            nc.tensor.matmul(
                dummy[:, :], blocksel[:, 56:120], blocksel[:, 0:8],
                start=True, stop=True,
            )
            psum = pspool.tile([SPC, 2 * RES], dt.float32)

            # persistent DVE-only work tiles: allocated once, reused across
            # groups -> plain same-engine ordering, no pool-release sems
            y = cpool.tile([NPART, 1152], dt.float32)
            ki = cpool.tile([NPART, 1152], dt.int32)
            yt = cpool.tile([NPART, 1152], dt.float32)
            m = cpool.tile([NPART, 1152], dt.float32)
            kcat = cpool.tile([NPART, KCAT_W], dt.bfloat16)
            scr = cpool.tile([NPART, 4096], dt.bfloat16)
            # +1 for V|Q cells, -1 for Eh|Ev cells
            sgn = cpool.tile([NPART, 4096], dt.bfloat16)
            nc.vector.memset(sgn[:, 0:2048], 1.0)
            nc.vector.memset(sgn[:, 2048:4096], -1.0)
            # per-threshold ACT bias constants: column t holds -(t + 0.5)
            bias = cpool.tile([NPART, RES], dt.float32)
            for t in range(TSPLIT, RES - 1):
                nc.vector.memset(bias[:, t : t + 1], -(t + 0.5))

            xf2 = None
            for g in range(GROUPS):
                if g % 2 == 0:
                    # one DMA loads two groups (6 DMAs total <= 8 HW lanes,
                    # so no lane-recycle waits anywhere)
                    xf2 = xfpool.tile([NPART, 2304], dt.float32, tag="xf")
                    nc.sync.dma_start(xf2[:, :], x_dram[g // 2])
                xf = xf2[:, (g % 2) * 1152 : (g % 2) * 1152 + 1152]

                nc.vector.tensor_scalar_mul(y[:, :], xf, 63.0)
                nc.vector.tensor_copy(ki[:, :], y[:, :])     # f32 -> int32 cast
                nc.vector.tensor_copy(yt[:, :], ki[:, :])    # int32 -> f32 cast
                nc.vector.tensor_tensor(m[:, :], y[:, :], yt[:, :], alu.is_gt)
                nc.vector.tensor_tensor(
                    kcat[:, VOFF : VOFF + 1024], yt[:, 0:1024], m[:, 0:1024], alu.add
                )
                nc.vector.tensor_tensor(
                    kcat[:, KAUX : KAUX + 128], yt[:, 1024:1152], m[:, 1024:1152],
                    alu.add,
                )

                k3 = kcat[:, VOFF : VOFF + 1024].rearrange("p (r w) -> p r w", w=W)
                kaux = kcat[:, KAUX : KAUX + 128]
                eh3 = kcat[:, EHOFF : EHOFF + 1024].rearrange("p (r w) -> p r w", w=W)
                ehaux = kcat[:, EHAUX : EHAUX + 128]
                ev3 = kcat[:, EVOFF : EVOFF + 1024].rearrange("p (r w) -> p r w", w=W)
                q3 = kcat[:, QOFF : QOFF + 1024].rearrange("p (r w) -> p r w", w=W)

                # ---- neighbor maxes (bf16, exact) ----
                # Eh rows 0..7 + aux row 8
                nc.vector.tensor_tensor(
                    eh3[:, :, 0:127], k3[:, :, 0:127], k3[:, :, 1:128], alu.max
                )
                nc.vector.tensor_tensor(
                    ehaux[:, 0:127], kaux[:, 0:127], kaux[:, 1:128], alu.max
                )
                nc.vector.memset(eh3[:, :, 127:128], 64.0)
                # Ev rows 0..6 bulk + row 7 vs aux
                nc.vector.tensor_tensor(
                    ev3[:, 0:7, :], k3[:, 0:7, :], k3[:, 1:8, :], alu.max
                )
                nc.vector.tensor_tensor(
                    ev3[:, 7, :], k3[:, 7, :], kaux[:, :], alu.max
                )
                # Q rows 0..6 bulk + row 7 vs Eh aux
                nc.vector.tensor_tensor(
                    q3[:, 0:7, 0:127], eh3[:, 0:7, 0:127], eh3[:, 1:8, 0:127], alu.max
                )
                nc.vector.tensor_tensor(
                    q3[:, 7, 0:127], eh3[:, 7, 0:127], ehaux[:, 0:127], alu.max
                )
                nc.vector.memset(q3[:, :, 127:128], 64.0)

                # ---- threshold loop: one signed fused pass per threshold ----
                # accum[p] = sum_cells sign * [K <= t]  (chi contribution).
                # t = RES-1 is skipped: every cell has K <= 63 (x < 1), so
                # chi(63) = V - Eh - Ev + Q = 1 exactly; host fills it in.
                cnt = cntpool.tile([NPART, 2 * RES], dt.float32, tag="cnt")
                # claim cnt (waits PE release) before the threshold loop
                nc.vector.memset(cnt[:, 0:4], 0.0)
                nc.vector.memset(cnt[:, RES - 1 : RES], 0.0)
                nc.vector.memset(cnt[:, 2 * RES - 1 : 2 * RES], 0.0)
                # DVE handles t in [0, TSPLIT); ACT (Scalar) handles the rest
                # via accum of Sign(K - t - 0.5) over pos / neg cells:
                # chi(t) = (S_neg - S_pos) / 2 (host decodes).
                for t in range(TSPLIT):
                    nc.vector.scalar_tensor_tensor(
                        scr[:, :], kcat[:, 0:4096], float(t), sgn[:, :],
                        alu.is_le, alu.mult,
                        accum_out=cnt[:, t : t + 1],
                    )
                for t in range(TSPLIT, RES - 1):
                    nc.scalar.activation(
                        scr[:, 0:2048], kcat[:, 0:2048],
                        mybir.ActivationFunctionType.Sign,
                        bias=bias[:, t : t + 1], scale=1.0,
                        accum_out=cnt[:, t : t + 1],
                    )
                    nc.scalar.activation(
                        scr[:, 2048:4096], kcat[:, 2048:4096],
                        mybir.ActivationFunctionType.Sign,
                        bias=bias[:, t : t + 1], scale=1.0,
                        accum_out=cnt[:, RES + t : RES + t + 1],
                    )

                # ---- per-slice reduction across partitions on PE ----
                # psum[8g + j, :] += per-slice-j sums of this group
                nc.tensor.matmul(
                    psum[:, :],
                    blocksel[:, 56 - 8 * g : 120 - 8 * g],
                    cnt[:, :],
                    start=(g == 0),
                    stop=(g == GROUPS - 1),
                )

            outt = cpool.tile([SPC, 2 * RES], dt.float32)
            nc.vector.tensor_copy(outt[:, :], psum[:, :])
            nc.sync.dma_start(out_dram[:, :], outt[:, :])

    if legalize:
        _legalize_waits(nc)
    return nc


def _legalize_waits(nc, max_waits: int = 1):
    """This walrus build rejects instructions with more than one sync wait.
    Split excess waits onto preceding same-engine NoOps."""
    import concourse.mybir as mybir

    for f in nc.m.functions:
        for b in f.blocks:
            il = list(b.instructions)
            out, changed = [], False
            for inst in il:
                try:
                    si = inst.sync_info
                except AttributeError:
                    si = None
                waits = list(si.on_wait) if si else []
                if len(waits) > max_waits:
                    head, keep = waits[:-max_waits], waits[-max_waits:]
                    for k, w in enumerate(head):
                        out.append(
                            mybir.InstNoOp(
                                name=f"{inst.name}-w{k}",
                                engine=inst.engine,
                                sync_info=mybir.SyncInfo(on_wait=[w], on_update=[]),
                                bass_nofuse=True,
                            )
                        )
                    inst.sync_info = mybir.SyncInfo(
                        on_wait=keep, on_update=list(si.on_update)
                    )
                    changed = True
                out.append(inst)
            if changed:
                b.instructions = out


def make_host_inputs(xcore: np.ndarray):
    """xcore [NROWS, W] f32 -> (xi, bsel) host-side input arrays."""
    x3 = xcore.reshape(GROUPS, NPART, 1024)
    xi = np.empty((GROUPS, NPART, 1152), dtype=np.float32)
    xi[:, :, 0:1024] = x3
    # overlap row: image row 8*(k+1) of the group
    xi[:, :127, 1024:1152] = x3[:, 1:, 0:128]
    xi[:, 127, 1024:1152] = 0.0
    for g in range(GROUPS - 1):
        xi[g, 127, 1024:1152] = x3[g + 1, 0, 0:128]
    xi[:, 15::16, 1024:1152] = 2.0  # slice-boundary phantoms
    # pack group pairs side by side per partition
    xi = np.ascontiguousarray(
        xi.reshape(GROUPS // 2, 2, NPART, 1152).transpose(0, 2, 1, 3)
    ).reshape(GROUPS // 2, NPART, 2304)
    bsel = np.zeros((NPART, 120), dtype=np.float32)
    for p in range(NPART):
        bsel[p, 56 + p // 16] = 1.0
    return xi, bsel


def _install_ntff_hook():
    """Provide antenv.axon_hooks (absent in this container) so
    run_bass_kernel_spmd(trace=True) can capture NTFF profiles."""
    import sys, types

    if "antenv.axon_hooks" in sys.modules:
        return
    mod = types.ModuleType("antenv.axon_hooks")
    state = {"hook": None}
    mod.set_axon_ntff_profile_hook = lambda h: state.update(hook=h)
    mod.get_axon_ntff_profile_hook = lambda: state["hook"]
    sys.modules["antenv.axon_hooks"] = mod
    try:
        from trn_agent_boot.trn_boot import _ntff_profile_via_ctypes

        hook = _ntff_profile_via_ctypes("/opt/axon/libaxon_pjrt.so")
        if hook is not None:
            mod.set_axon_ntff_profile_hook(hook)
    except Exception:
        pass


def _run(x: np.ndarray, trace: bool = False):
    from concourse import bass_utils

    if trace:
        _install_ntff_hook()

    x = np.ascontiguousarray(np.asarray(x), dtype=np.float32)
    assert x.shape == (B, C, H, W)

    if "nc" not in _CACHE:
        _CACHE["nc"] = _build_program()
    nc = _CACHE["nc"]

    flat = x.reshape(NCORES, NROWS, W)
    in_maps = []
    for k in range(NCORES):
        xi, bsel = make_host_inputs(flat[k])
        in_maps.append({"xi": xi, "bsel": bsel})
    res = bass_utils.run_bass_kernel_spmd(
        nc, in_maps, core_ids=list(range(NCORES)), trace=trace
    )
    outs = [r["cnt"] for r in res.results]  # each [SPC, 2*RES] f32
    cnt = np.stack(outs, axis=0).reshape(SLICES, 2 * RES)
    ecc = cnt[:, 0:RES].copy()
    # ACT-computed thresholds: chi = (S_neg - S_pos) / 2
    for t in range(TSPLIT, RES - 1):
        ecc[:, t] = (cnt[:, RES + t] - cnt[:, t]) / 2.0
    # chi at the top threshold is the Euler characteristic of the full square
    ecc[:, RES - 1] = 1.0
    return ecc.reshape(B, C, RES).astype(np.float32), res


def kernel(x: np.ndarray) -> np.ndarray:
    out, _ = _run(x, trace=False)
    return out



# revision 9
# speedup vs baseline: 4.3463x; 4.3463x over previous
"""Euler characteristic curve (cubical complex) kernel for Trainium2.

Problem: x [32,16,128,128] f32 -> ECC [32,16,64] f32.
Per (b,c) slice: every cell of the 255x255 vertex-mode cubical grid has
filtration bin K = ceil(63*max(corner values)) in [0,63];
ECC(t) = #V(K<=t) - #Eh(K<=t) - #Ev(K<=t) + #Q(K<=t).

Strategy (per core, 64 slices, pure data parallel over 8 cores):
 - Lower-star compression: assign each cell to the lexicographically-first
   corner achieving its max bin. Every cell assigned to vertex v activates at
   bin K_v, so chi(t) = sum_v w_v * [K_v <= t] with a t-independent integer
   weight w_v = 1 - (#edges assigned) + (#squares assigned) in [-3, 1].
   This turns 4 cell arrays into ONE weighted vertex array for the DVE.
 - Layout: partition p = (slice s=p//2, half h=p%2); each partition holds 64
   image rows + up/down overlap rows, row stride 130 (128 cols + 2 sentinel
   cols, sentinel value 1024 in bin domain > 63 self-excludes from counts and
   kills cross-boundary cells in the beat comparisons).
 - Exact binning: y=63*x on ACT (exact fp32 FMA), int-cast ceil trick on DVE.
 - Per DVE threshold t: mask = (K is_le t) @4x, mask*w @2x, PE column-sum
   matmuls (blocksel weights, 17 chunks) accumulate per-slice sums in PSUM,
   ACT Copy+accum_out tail-reduces PSUM -> chi[:, t]. Exact int arithmetic.
 - Remaining thresholds on ACT: Sign(X - t - .5) + accum over the 4 plain
   cell arrays (V=K, Eh, Ev, Q maxes); host decodes counts from sign-sums.
 - t = 63: chi = Euler characteristic of the full square = 1 (host constant).
"""

import numpy as np

B, C, H, W = 32, 16, 128, 128
RES = 64
NCORES = 8
SLICES = B * C              # 512
SPC = SLICES // NCORES      # 64 slices per core
NPART = 128

SW = 130                    # row stride: 128 cols + 2 sentinel columns
ROWS = 67                   # pad row + up-overlap + 64 owned + down-overlap
WTOT = ROWS * SW            # 8710 input width per partition
KW = WTOT + 4               # K tile width (pad, memset to sentinel)
OWN = 260                   # owned rows start (flat offset, row 2)
OWN_W = 64 * SW             # 8320 owned width
EHX_W = 8582                # Ehx width (Eh over flat 129..8710, padded even)
XSENT = 20.25               # x-domain sentinel -> K = ceil(63*20.25) = 1276
NMM = 17                    # 16x512 + 1x128 moving chunks per threshold

N_DVE = 50                  # thresholds on the DVE/PE pipeline (t = 0..N_DVE-1)
N_ACT = RES - 1 - N_DVE     # thresholds on ACT (t = N_DVE..62)

_CACHE = {}


def _build_program(legalize=True):
    import concourse.bass as bass
    import concourse.mybir as mybir
    from concourse.tile import TileContext
    from concourse.alu_op_type import AluOpType as alu

    dt = mybir.dt
    af = mybir.ActivationFunctionType
    nc = bass.Bass("TRN2", target_bir_lowering=False, debug=False)

    x_dram = nc.dram_tensor("xi", [NPART, WTOT], dt.float32, kind="ExternalInput").ap()
    bsel_dram = nc.dram_tensor("bsel", [NPART, 64], dt.float32, kind="ExternalInput").ap()
    bias_dram = nc.dram_tensor("bias", [NPART, 64], dt.float32, kind="ExternalInput").ap()
    chi_dram = nc.dram_tensor("chi", [SPC, 64], dt.float32, kind="ExternalOutput").ap()
    acts_dram = nc.dram_tensor("acts", [NPART, 4 * N_ACT], dt.float32, kind="ExternalOutput").ap()

    HCH = 4356  # K-compute column chunk width (even; chunks 4356 + 4354)

    with TileContext(nc) as tc:
        with (
            tc.tile_pool(name="persist", bufs=1) as ap_,
            tc.tile_pool(name="ps", bufs=4, space="PSUM") as pp,
        ):
            # ---- persistent tiles ----
            K = ap_.tile([NPART, KW], dt.bfloat16)
            w = ap_.tile([NPART, OWN_W], dt.bfloat16)
            ehx = ap_.tile([NPART, EHX_W], dt.bfloat16)
            q = ap_.tile([NPART, OWN_W], dt.bfloat16)
            bself = ap_.tile([NPART, 64], dt.float32)
            bselb = ap_.tile([NPART, 64], dt.bfloat16)
            biasT = ap_.tile([NPART, 64], dt.float32)
            chi = ap_.tile([SPC, 64], dt.float32)
            acts = ap_.tile([NPART, 4 * N_ACT], dt.float32)
            scr512 = ap_.tile([SPC, 512], dt.bfloat16)
            ascr = ap_.tile([NPART, OWN_W], dt.bfloat16)

            nc.sync.dma_start(bself[:, :], bsel_dram)
            nc.sync.dma_start(biasT[:, :], bias_dram)
            nc.vector.tensor_copy(bselb[:, :], bself[:, :])
            nc.vector.memset(K[:, WTOT:KW], 1276.0)

            # ---- K = ceil(63*x), exact (ACT mult + int-cast ceil) ----
            with tc.tile_pool(name="kprep", bufs=1) as kp:
                xf = kp.tile([NPART, WTOT], dt.float32)
                ft = kp.tile([NPART, 3 * HCH], dt.float32)
                ht = kp.tile([NPART, 2 * HCH], dt.bfloat16)
                nc.sync.dma_start(xf[:, 0:HCH], x_dram[:, 0:HCH])
                nc.sync.dma_start(xf[:, HCH:WTOT], x_dram[:, HCH:WTOT])
                for lo, hi in ((0, HCH), (HCH, WTOT)):
                    cw = hi - lo
                    y = ft[:, 0:cw]
                    yt = ft[:, HCH : HCH + cw]
                    ki = ft[:, 2 * HCH : 2 * HCH + cw].bitcast(dt.int32)
                    de = ht[:, 0:cw]
                    ytb = ht[:, HCH : HCH + cw]
                    nc.scalar.activation(y, xf[:, lo:hi], af.Copy, bias=0.0, scale=63.0)
                    nc.vector.tensor_copy(ki, y)                      # f32 -> int32
                    nc.vector.tensor_copy(yt, ki)                     # int32 -> f32
                    nc.vector.tensor_tensor(de, y, yt, alu.is_gt)     # 1x f32
                    nc.vector.tensor_copy(ytb, yt)                    # f32 -> bf16
                    nc.vector.tensor_tensor(K[:, lo:hi], de, ytb, alu.add)

            # ---- Eh array (flat positions 129 .. 129+EHX_W) and Q ----
            nc.vector.tensor_tensor(
                ehx[:, :], K[:, 129 : 129 + EHX_W], K[:, 130 : 130 + EHX_W], alu.max
            )
            nc.vector.tensor_tensor(
                q[:, :], ehx[:, 131 : 131 + OWN_W], ehx[:, 261 : 261 + OWN_W], alu.max
            )

            Ko = K[:, OWN : OWN + OWN_W]

            # ---- per-vertex weights w = 1 - E + S (lex-first tie-break) ----
            with tc.tile_pool(name="wprep", bufs=1) as wp:
                tt = wp.tile([NPART, 6 * OWN_W], dt.bfloat16)
                s = [tt[:, i * OWN_W : (i + 1) * OWN_W] for i in range(6)]
                TT = nc.vector.tensor_tensor
                TT(s[0], Ko, K[:, OWN - 1 : OWN - 1 + OWN_W], alu.is_gt)    # bL'
                TT(s[1], Ko, K[:, OWN + 1 : OWN + 1 + OWN_W], alu.is_ge)    # bR'
                TT(s[2], Ko, K[:, OWN - SW : OWN - SW + OWN_W], alu.is_gt)  # bU'
                TT(s[3], Ko, K[:, OWN + SW : OWN + SW + OWN_W], alu.is_ge)  # bD'
                TT(s[4], s[0], s[1], alu.add)                               # e1
                TT(s[5], s[2], s[3], alu.add)                               # e2
                TT(s[2], s[4], s[5], alu.add)                               # E
                TT(s[3], Ko, ehx[:, 0:OWN_W], alu.is_gt)                    # cUL
                TT(s[4], s[3], s[0], alu.mult)                              # S_ul
                TT(s[5], Ko, ehx[:, 1 : 1 + OWN_W], alu.is_gt)              # cUR
                TT(s[3], s[5], s[1], alu.mult)                              # S_ur
                TT(s[5], Ko, ehx[:, 260 : 260 + OWN_W], alu.is_ge)          # cLL
                TT(s[1], s[5], s[0], alu.mult)                              # S_ll
                TT(s[5], Ko, q[:, :], alu.is_ge)                            # S_lr
                TT(s[0], s[4], s[3], alu.add)                               # S_ul+S_ur
                TT(s[3], s[1], s[5], alu.add)                               # S_ll+S_lr
                TT(s[4], s[0], s[3], alu.add)                               # S
                nc.vector.tensor_scalar(s[5], s[2], -1.0, 1.0, alu.mult, alu.add)  # 1-E
                TT(w[:, :], s[4], s[5], alu.add)                            # w

            # ---- threshold loop ----
            with tc.tile_pool(name="thr", bufs=3) as mp:
                ev = None
                ia = 0
                for t in range(RES - 1):
                    if (t % 5 == 4 and ia < N_ACT) or (RES - 1 - t <= N_ACT - ia):
                        # ACT route: sign-count the 4 plain cell arrays
                        if ev is None:
                            ev = ap_.tile([NPART, OWN_W], dt.bfloat16)
                            nc.vector.tensor_tensor(
                                ev[:, :], Ko, K[:, OWN + SW : OWN + SW + OWN_W], alu.max
                            )
                        arrs = (Ko, ehx[:, 131 : 131 + OWN_W], ev[:, :], q[:, :])
                        for j, arr in enumerate(arrs):
                            nc.scalar.activation(
                                ascr[:, :], arr, af.Sign,
                                bias=biasT[:, t : t + 1], scale=1.0,
                                accum_out=acts[:, 4 * ia + j : 4 * ia + j + 1],
                            )
                        ia += 1
                        continue
                    # DVE route
                    m0 = mp.tile([NPART, OWN_W], dt.bfloat16, tag="m0")
                    m1 = mp.tile([NPART, OWN_W], dt.bfloat16, tag="m1")
                    nc.vector.tensor_scalar(m0[:, :], Ko, float(t), None, alu.is_le)
                    nc.vector.tensor_tensor(m1[:, :], m0[:, :], w[:, :], alu.mult)
                    psum = pp.tile([SPC, 512], dt.float32, tag="ps")
                    for c in range(NMM):
                        lo = 512 * c
                        hi = min(512 * (c + 1), OWN_W)
                        nc.tensor.matmul(
                            psum[:, 0 : hi - lo], bselb[:, :], m1[:, lo:hi],
                            start=(c == 0), stop=(c == NMM - 1),
                        )
                    nc.scalar.activation(
                        scr512[:, :], psum[:, :], af.Copy, bias=0.0, scale=1.0,
                        accum_out=chi[:, t : t + 1],
                    )

            nc.sync.dma_start(chi_dram, chi[:, :])
            nc.sync.dma_start(acts_dram, acts[:, :])

    if legalize:
        _legalize_waits(nc)
    return nc


def _legalize_waits(nc, max_waits: int = 1):
    """This walrus build rejects instructions with more than one sync wait.
    Split excess waits onto preceding same-engine NoOps."""
    import concourse.mybir as mybir

    for f in nc.m.functions:
        for b in f.blocks:
            il = list(b.instructions)
            out, changed = [], False
            for inst in il:
                try:
                    si = inst.sync_info
                except AttributeError:
                    si = None
                waits = list(si.on_wait) if si else []
                if len(waits) > max_waits:
                    head, keep = waits[:-max_waits], waits[-max_waits:]
                    for k, wv in enumerate(head):
                        out.append(
                            mybir.InstNoOp(
                                name=f"{inst.name}-w{k}",
                                engine=inst.engine,
                                sync_info=mybir.SyncInfo(on_wait=[wv], on_update=[]),
                                bass_nofuse=True,
                            )
                        )
                    inst.sync_info = mybir.SyncInfo(
                        on_wait=keep, on_update=list(si.on_update)
                    )
                    changed = True
                out.append(inst)
            if changed:
                b.instructions = out


def make_host_inputs(xcore: np.ndarray):
    """xcore [SPC, H, W] f32 -> packed xi [NPART, WTOT] plus bsel/bias."""
    xi = np.full((SPC, 2, ROWS, SW), XSENT, dtype=np.float32)
    xh = xcore.reshape(SPC, 2, 64, W)
    xi[:, :, 2:66, 0:W] = xh
    xi[:, 1, 1, 0:W] = xcore[:, 63, :]   # h=1 up-overlap = image row 63
    xi[:, 0, 66, 0:W] = xcore[:, 64, :]  # h=0 down-overlap = image row 64
    return xi.reshape(NPART, WTOT)


def _host_bsel_bias():
    bsel = np.zeros((NPART, 64), dtype=np.float32)
    bsel[np.arange(NPART), np.arange(NPART) // 2] = 1.0
    bias = np.broadcast_to(
        -(np.arange(64, dtype=np.float32) + 0.5), (NPART, 64)
    ).copy()
    return bsel, bias


def _install_ntff_hook():
    import sys, types

    if "antenv.axon_hooks" in sys.modules:
        return
    mod = types.ModuleType("antenv.axon_hooks")
    state = {"hook": None}
    mod.set_axon_ntff_profile_hook = lambda h: state.update(hook=h)
    mod.get_axon_ntff_profile_hook = lambda: state["hook"]
    sys.modules["antenv.axon_hooks"] = mod
    try:
        from trn_agent_boot.trn_boot import _ntff_profile_via_ctypes

        hook = _ntff_profile_via_ctypes("/opt/axon/libaxon_pjrt.so")
        if hook is not None:
            mod.set_axon_ntff_profile_hook(hook)
    except Exception:
        pass


def _act_threshold_list():
    ts, ia = [], 0
    for t in range(RES - 1):
        if (t % 5 == 4 and ia < N_ACT) or (RES - 1 - t <= N_ACT - ia):
            ts.append(t)
            ia += 1
    return ts


def _run(x: np.ndarray, trace: bool = False):
    from concourse import bass_utils

    if trace:
        _install_ntff_hook()

    x = np.ascontiguousarray(np.asarray(x), dtype=np.float32)
    assert x.shape == (B, C, H, W)

    if "nc" not in _CACHE:
        _CACHE["nc"] = _build_program()
    nc = _CACHE["nc"]

    bsel, bias = _host_bsel_bias()
    flat = x.reshape(SLICES, H, W)
    in_maps = []
    for k in range(NCORES):
        xi = make_host_inputs(flat[k * SPC : (k + 1) * SPC])
        in_maps.append({"xi": xi, "bsel": bsel, "bias": bias})
    res = bass_utils.run_bass_kernel_spmd(
        nc, in_maps, core_ids=list(range(NCORES)), trace=trace
    )

    act_ts = _act_threshold_list()
    ecc = np.empty((SLICES, RES), dtype=np.float64)
    for k in range(NCORES):
        chi = res.results[k]["chi"]          # [SPC, 64] f32
        acts = res.results[k]["acts"]        # [NPART, 4*N_ACT] f32
        sl = slice(k * SPC, (k + 1) * SPC)
        ecc[sl, : RES - 1] = chi[:, : RES - 1].astype(np.float64)
        a = acts.astype(np.float64).reshape(SPC, 2, N_ACT, 4).sum(axis=1)
        # chi = (aEh + aEv - aV - aQ) / 2 summed over the two halves
        for ia, t in enumerate(act_ts):
            ecc[sl, t] = (a[:, ia, 1] + a[:, ia, 2] - a[:, ia, 0] - a[:, ia, 3]) / 2.0
    ecc[:, RES - 1] = 1.0
    return ecc.reshape(B, C, RES).astype(np.float32), res


def kernel(x: np.ndarray) -> np.ndarray:
    out, _ = _run(x, trace=False)
    return out
